# revision 20
# baseline (speedup 1.0000x reference)
"""Sliding-window causal self-attention (GQA + RoPE + QK-RMSNorm + ve-gate) on
8 Trainium2 NeuronCores.

Sharding: core c handles (batch b = c // 4, kv-head g = c % 4): data parallel
over batch x tensor parallel over the 4 KV head groups (4 query heads per
core). Each core computes its partial c_proj output; the all-reduce over the 4
head shards is a host-side sum.

Device design (per core):
  - x is fed transposed (xT: C x T) in bf16 so all projections contract over
    the partition axis at the full 1-col/cycle PE rate.
  - q, k are built transposed (qT/kT: head-dim x T); scores are computed
    TRANSPOSED (S^T: tk x tq) so softmax denominators come from a ones-matmul
    on the Tensor engine and P@V needs no transposes.
  - softmax skips max-subtraction: QK RMS-norm bounds |scores| <= 1.44*sqrt(128)
    so exp() cannot overflow. Masking multiplies the two triangular boundary
    blocks by {0,1} masks after exp.
  - k's rms-norm scale rides the per-partition `scale` operand of the Exp
    activation; q's rides the PSUM-evacuation multiply.
  - all matmuls run in bf16 (flat 1 col/cycle on the PE regardless of moving
    size); PSUM accumulation is fp32.
"""

import sys

sys.path.insert(0, "/opt/trn_rl_repo")

import numpy as np

B, T, C = 2, 2048, 2048
NH, NKV, HD = 16, 4, 128
GATE_CH = 12
HPC = NH // NKV          # q heads per core
TS = 512                 # token-slice width
NSL = T // TS            # 4 slices
NCK = C // 128           # 16 contraction chunks
TPS = TS // 128          # 4 token tiles per slice
NTT = T // 128           # 16 token tiles
EPS = 1e-6
NEG = -100.0

A_Q = 1.2 / np.sqrt(float(HD))   # rms-norm scale folded into q (incl 1/sqrt(HD))
A_K = 1.2                        # rms-norm scale folded into exp() scale arg
S_Q = float(1.0 / (HD * A_Q * A_Q))
B_Q = float(EPS / (A_Q * A_Q))
S_K = float(1.0 / (HD * A_K * A_K))
B_K = float(EPS / (A_K * A_K))

_compiled = {}


def _ktiles(m4, W):
    """k-tiles overlapping q-slice m4 with their valid tq-column extents.

    Returns list of (n, f0, f1, causal_block_col, edge_block_col); columns are
    relative to the slice (0..TS). First entry covers [0, TS) fully (it opens
    the PSUM accumulation group).
    """
    assert W % 128 == 0 and W >= 384
    out = []
    for n in range(0, TPS * m4 + TPS):
        f0 = max(0, 128 * n - TS * m4)
        f1 = min(TS, 128 * n + W + 128 - TS * m4)
        if f1 <= f0:
            continue
        causal = 128 * n >= TS * m4            # diagonal staircase inside tile
        edge = (128 * n + W + 128 - TS * m4) <= TS  # window lower edge inside
        cb = f0 if causal else None
        eb = (f1 - 128) if edge else None
        out.append((n, f0, f1, cb, eb))
    full = [e for e in out if e[1] == 0 and e[2] == TS]
    assert full, "need one full-extent tile to open the PSUM group"
    first = full[0]
    rest = [e for e in out if e[0] != first[0]]
    return [first] + rest


def _build(W):
    import concourse.bass as bass
    import concourse.tile as tile
    from concourse import bacc, mybir
    from concourse.masks import make_identity
    from contextlib import ExitStack

    f32 = mybir.dt.float32
    f32r = mybir.dt.float32r
    bf16 = mybir.dt.bfloat16
    AF = mybir.ActivationFunctionType
    OP = mybir.AluOpType

    nc = bacc.Bacc(None, target_bir_lowering=False)

    xT = nc.dram_tensor("xT", [C, T], bf16, kind="ExternalInput")
    wq = nc.dram_tensor("wqT", [C, HPC * HD], bf16, kind="ExternalInput")
    wk = nc.dram_tensor("wkT", [C, HD], bf16, kind="ExternalInput")
    wv = nc.dram_tensor("wvT", [C, HD], bf16, kind="ExternalInput")
    wp = nc.dram_tensor("wpT", [HPC * HD, C], bf16, kind="ExternalInput")
    wgd = nc.dram_tensor("wg", [GATE_CH, 1], bf16, kind="ExternalInput")
    csd = nc.dram_tensor("cs", [HD, 2, T], bf16, kind="ExternalInput")
    ved = nc.dram_tensor("ve", [T, HD], bf16, kind="ExternalInput")
    btrid = nc.dram_tensor("btri", [128, 128], bf16, kind="ExternalInput")
    etrid = nc.dram_tensor("etri", [128, 128], bf16, kind="ExternalInput")
    outT = nc.dram_tensor("outT", [C, T], bf16, kind="ExternalOutput")

    with tile.TileContext(nc) as tc, ExitStack() as ctx:
        res = ctx.enter_context(tc.tile_pool(name="res", bufs=1))
        xc_p = ctx.enter_context(tc.tile_pool(name="xc", bufs=2))
        tab_p = ctx.enter_context(tc.tile_pool(name="tab", bufs=2))
        work_p = ctx.enter_context(tc.tile_pool(name="work", bufs=2))
        sq_p = ctx.enter_context(tc.tile_pool(name="sq", bufs=3))
        bc_p = ctx.enter_context(tc.tile_pool(name="bc", bufs=2))
        qt_p = ctx.enter_context(tc.tile_pool(name="qt", bufs=2))
        es_p = ctx.enter_context(tc.tile_pool(name="es", bufs=6))
        yt_p = ctx.enter_context(tc.tile_pool(name="yt", bufs=1))
        ot_p = ctx.enter_context(tc.tile_pool(name="ot", bufs=3))
        row_p = ctx.enter_context(tc.tile_pool(name="rows", bufs=1))

        ps_qkv = ctx.enter_context(tc.tile_pool(name="ps_qkv", bufs=2, space="PSUM"))
        ps_s = ctx.enter_context(tc.tile_pool(name="ps_s", bufs=3, space="PSUM"))
        ps_out_p = ctx.enter_context(tc.tile_pool(name="ps_out", bufs=2, space="PSUM"))
        ps_row = ctx.enter_context(tc.tile_pool(name="ps_row", bufs=1, space="PSUM"))
        dram_p = ctx.enter_context(tc.tile_pool(name="dram", bufs=2, space="DRAM"))

        # resident tensors; weight loads split per chunk so the first QKV
        # matmuls can start as soon as their chunk lands (startup latency).
        wq_sb = res.tile([128, NCK, HPC * HD], bf16)
        wk_sb = res.tile([128, NCK, HD], bf16)
        wv_sb = res.tile([128, NCK, HD], bf16)
        wp_sb = res.tile([128, HPC, C], bf16)   # loaded later, before cproj(0)
        wg_sb = res.tile([GATE_CH, 1], bf16)
        nc.sync.dma_start(out=wg_sb, in_=wgd[:, :])
        btri_sb = res.tile([128, 128], bf16)   # -100/0 bias, transposed (lhsT)
        nc.sync.dma_start(out=btri_sb, in_=btrid[:, :])
        etri_sb = res.tile([128, 128], bf16)
        nc.sync.dma_start(out=etri_sb, in_=etrid[:, :])
        ident = res.tile([128, 128], f32)
        make_identity(nc, ident)
        ident_bf = res.tile([128, 128], bf16)
        make_identity(nc, ident_bf)
        ones_sb = res.tile([128, 1], bf16)
        nc.vector.memset(ones_sb, 1.0)
        bq_sb = res.tile([1, 1], f32)
        nc.vector.memset(bq_sb, B_Q)
        bk_sb = res.tile([128, 1], f32)
        nc.vector.memset(bk_sb, B_K)
        kT_sb = res.tile([128, T], bf16)        # rotated k, head-dim on partitions
        vn_sb = res.tile([128, NTT, HD], bf16)  # v natural, token tiles on partitions
        rnk_sb = res.tile([128, NTT], f32)      # per-k-tile rms-norm columns

        def rope_inplace(dst, cc_sl, ss_sl):
            """dst (128, TS) bf16 holding pre-rotation values. In-place RoPE."""
            qsw = work_p.tile([128, TS], bf16, tag="qsw")
            nc.sync.dma_start(out=qsw[0:64, :], in_=dst[64:128, :])
            nc.sync.dma_start(out=qsw[64:128, :], in_=dst[0:64, :])
            tmp = work_p.tile([128, TS], bf16, tag="tmp")
            nc.vector.tensor_mul(tmp, qsw, ss_sl)
            nc.vector.tensor_mul(dst, dst, cc_sl)
            nc.vector.tensor_add(dst, dst, tmp)

        for m4 in range(NSL):
            t0 = m4 * TS
            # ---- stream x slice + tables ----
            xca = xc_p.tile([128, NCK, TS], bf16, tag="xc")
            xsrc = xT[:, t0:t0 + TS].rearrange("(ck p) t -> p ck t", p=128)
            if m4 == 0:
                # chunk 0 lands first so the gate/k matmuls start early
                nc.sync.dma_start(out=xca[:, 0:1, :], in_=xsrc[:, 0:1, :])
                nc.sync.dma_start(out=wk_sb[:, :, :],
                                  in_=wk.rearrange("(ck p) h -> p ck h", p=128))
                nc.sync.dma_start(out=xca[:, 1:8, :], in_=xsrc[:, 1:8, :])
                nc.sync.dma_start(out=xca[:, 8:NCK, :], in_=xsrc[:, 8:NCK, :])
            else:
                nc.sync.dma_start(out=xca[:, 0:8, :], in_=xsrc[:, 0:8, :])
                nc.sync.dma_start(out=xca[:, 8:NCK, :], in_=xsrc[:, 8:NCK, :])
            xc = [xca[:, c, :] for c in range(NCK)]
            cs_sl = tab_p.tile([128, 2, TS], bf16, tag="cs")
            nc.sync.dma_start(out=cs_sl, in_=csd[:, :, t0:t0 + TS])
            cc_sl = cs_sl[:, 0, :]
            ss_sl = cs_sl[:, 1, :]
            ve_sl = tab_p.tile([128, TPS, HD], bf16, tag="ve")
            nc.sync.dma_start(
                out=ve_sl, in_=ved[t0:t0 + TS, :].rearrange("(tt p) h -> p tt h", p=128)
            )

            # ---- gate columns: 3*sigmoid(x[:, :12] @ wg) ----
            ps_g = ps_s.tile([1, TS], f32, tag="s")
            nc.tensor.matmul(ps_g, wg_sb, xc[0][0:GATE_CH, :], start=True, stop=True)
            g_row = row_p.tile([1, TS], f32, tag="grow")
            nc.scalar.activation(g_row, ps_g, AF.Exp, scale=-1.0)
            nc.vector.tensor_scalar(out=g_row, in0=g_row, scalar1=1.0, scalar2=None,
                                    op0=OP.add)
            nc.vector.reciprocal(g_row, g_row)
            g_dr = dram_p.tile([TS], f32, tag="gdr")
            nc.sync.dma_start(out=g_dr, in_=g_row)
            gate_c = row_p.tile([128, TPS], f32, tag="gate")
            nc.sync.dma_start(
                out=gate_c,
                in_=bass.AP(tensor=g_dr.tensor, offset=g_dr.offset,
                            ap=[[1, 128], [128, TPS]]),
            )

            # ---- k+v projections (chunk-interleaved so matmuls can retire
            # while the x slice is still streaming in) ----
            if m4 == 0:
                nc.sync.dma_start(out=wv_sb[:, :, :],
                                  in_=wv.rearrange("(ck p) h -> p ck h", p=128))
            ps_k = ps_qkv.tile([128, TS], f32, tag="qkv")
            ps_v = ps_qkv.tile([128, TS], f32, tag="qkv")
            for c in range(NCK):
                nc.tensor.matmul(ps_k, wk_sb[:, c, :], xc[c],
                                 start=(c == 0), stop=(c == NCK - 1))
                nc.tensor.matmul(ps_v, wv_sb[:, c, :], xc[c],
                                 start=(c == 0), stop=(c == NCK - 1))
            sq_k = sq_p.tile([128, TS], bf16, tag="sq")
            nc.scalar.activation(sq_k, ps_k, AF.Square)
            ps_rk = ps_s.tile([1, TS], f32, tag="s")
            nc.tensor.matmul(ps_rk, ones_sb, sq_k, start=True, stop=True)
            srk = row_p.tile([1, TS], f32, tag="srk")
            nc.scalar.activation(srk, ps_rk, AF.Ln, bias=bk_sb[0:1], scale=S_K)
            nc.scalar.activation(srk, srk, AF.Exp, scale=-0.5)
            k_dr = dram_p.tile([TS], f32, tag="kdr")
            nc.sync.dma_start(out=k_dr, in_=srk)
            nc.sync.dma_start(
                out=rnk_sb[:, m4 * TPS:(m4 + 1) * TPS],
                in_=bass.AP(tensor=k_dr.tensor, offset=k_dr.offset,
                            ap=[[1, 128], [128, TPS]]),
            )
            k_sl = kT_sb[:, t0:t0 + TS]
            nc.vector.tensor_copy(k_sl, ps_k)
            rope_inplace(k_sl, cc_sl, ss_sl)

            # ---- v transpose to natural + gate-add ----
            vT_s = work_p.tile([128, TS], f32, tag="vts")
            nc.vector.tensor_copy(vT_s, ps_v)
            for tt in range(TPS):
                ps_t = ps_s.tile([128, TS], f32, tag="s")
                nc.tensor.transpose(ps_t[:, 0:128], vT_s[:, tt * 128:(tt + 1) * 128],
                                    ident)
                gtmp = work_p.tile([128, HD], bf16, tag="gtmp")
                nc.vector.tensor_scalar(out=gtmp, in0=ve_sl[:, tt, :],
                                        scalar1=gate_c[:, tt:tt + 1], scalar2=3.0,
                                        op0=OP.mult, op1=OP.mult)
                nc.vector.tensor_add(vn_sb[:, m4 * TPS + tt, :], ps_t[:, 0:128], gtmp)

            # ---- q projections (4 heads) + rms-norm + rope ----
            if m4 == 0:
                wqsrc = wq.rearrange("(ck p) h -> p ck h", p=128)
                for cg in range(0, NCK, 4):
                    nc.sync.dma_start(out=wq_sb[:, cg:cg + 4, :],
                                      in_=wqsrc[:, cg:cg + 4, :])
            qts = []
            for h in range(HPC):
                ps_q = ps_qkv.tile([128, TS], f32, tag="qkv")
                for c in range(NCK):
                    nc.tensor.matmul(ps_q, wq_sb[:, c, h * HD:(h + 1) * HD], xc[c],
                                     start=(c == 0), stop=(c == NCK - 1))
                sq_q = sq_p.tile([128, TS], bf16, tag="sq")
                nc.scalar.activation(sq_q, ps_q, AF.Square)
                ps_r = ps_s.tile([1, TS], f32, tag="s")
                nc.tensor.matmul(ps_r, ones_sb, sq_q, start=True, stop=True)
                srow = row_p.tile([1, TS], f32, tag="srow")
                nc.scalar.activation(srow, ps_r, AF.Ln, bias=bq_sb, scale=S_Q)
                nc.scalar.activation(srow, srow, AF.Exp, scale=-0.5)
                rbc = bc_p.tile([128, TS], f32, tag="bc")
                nc.gpsimd.partition_broadcast(rbc, srow)
                qt = qt_p.tile([128, TS], bf16, tag=f"qt{h}")
                nc.vector.tensor_mul(qt, ps_q, rbc)
                rope_inplace(qt, cc_sl, ss_sl)
                qts.append(qt)

            # ---- attention (scores transposed: tk on partitions, tq free) ----
            tiles = _ktiles(m4, W)
            last = len(tiles) - 1
            yts = []
            for h in range(HPC):
                ps_out = ps_out_p.tile([128, TS], f32, tag="out")
                ps_sum = ps_row.tile([1, TS], f32, tag="rows")
                for idx, (n, f0, f1, cb, eb) in enumerate(tiles):
                    pss = ps_s.tile([128, TS], f32, tag="s")
                    nmask = (cb is not None) + (eb is not None)
                    nc.tensor.matmul(pss[:, f0:f1], kT_sb[:, n * 128:(n + 1) * 128],
                                     qts[h][:, f0:f1], start=True, stop=(nmask == 0))
                    # -100/0 masking bias accumulated straight into the scores
                    # psum (bias.T @ I); masked weights underflow to 0 in exp.
                    if cb is not None:
                        nc.tensor.matmul(pss[:, cb:cb + 128], btri_sb, ident_bf,
                                         start=False, stop=(eb is None))
                    if eb is not None:
                        nc.tensor.matmul(pss[:, eb:eb + 128], etri_sb, ident_bf,
                                         start=False, stop=True)
                    es = es_p.tile([128, TS], bf16, tag="es")
                    nc.scalar.activation(es[:, f0:f1], pss[:, f0:f1], AF.Exp,
                                         scale=rnk_sb[:, n:n + 1])
                    nc.tensor.matmul(ps_sum[:, f0:f1], ones_sb, es[:, f0:f1],
                                     start=(idx == 0), stop=(idx == last))
                    nc.tensor.matmul(ps_out[:, f0:f1], vn_sb[:, n, :], es[:, f0:f1],
                                     start=(idx == 0), stop=(idx == last))
                rsum = row_p.tile([1, TS], f32, tag="rsum")
                nc.vector.reciprocal(rsum, ps_sum)
                sbc = bc_p.tile([128, TS], f32, tag="bc")
                nc.gpsimd.partition_broadcast(sbc, rsum)
                yt = yt_p.tile([128, TS], bf16, tag=f"yt{h}")
                nc.vector.tensor_mul(yt, ps_out, sbc)
                yts.append(yt)

            # ---- c_proj partial: outT[co, t] = sum_h wpT[h].T @ yT[h] ----
            if m4 == 0:
                nc.sync.dma_start(out=wp_sb[:, :, :],
                                  in_=wp.rearrange("(h p) c -> p h c", p=128))
            for co4 in range(0, NTT, 4):
                ot = ot_p.tile([128, 4, TS], bf16, tag="ot")
                for ci in range(4):
                    co = co4 + ci
                    ps_p = ps_s.tile([128, TS], f32, tag="s")
                    for h in range(HPC):
                        nc.tensor.matmul(ps_p, wp_sb[:, h, co * 128:(co + 1) * 128],
                                         yts[h], start=(h == 0), stop=(h == HPC - 1))
                    if ci % 2 == 0:
                        nc.vector.tensor_copy(ot[:, ci, :], ps_p)
                    else:
                        nc.scalar.copy(ot[:, ci, :], ps_p)
                nc.sync.dma_start(
                    out=outT[co4 * 128:(co4 + 4) * 128,
                             t0:t0 + TS].rearrange("(ck p) t -> p ck t", p=128),
                    in_=ot)

    # Restrict the activation-table picker to the one set containing every
    # ACT function we use (exp, ln, square, copy, identity): without this the
    # greedy picker alternates exp_and_others <-> natural_log, inserting a
    # ~1.3us table load per switch. Set ids are positions in act_info.json's
    # list, so unwanted sets are emptied rather than removed.
    import concourse.hw_specs as hw_specs
    import concourse.bacc as bacc_mod

    orig = hw_specs.get_activation_tables

    def only_combined(arch):
        t = orig(arch)
        return {k: (v if k == "natural_log_exp_and_others" else set())
                for k, v in t.items()}

    hw_specs.get_activation_tables = only_combined
    bacc_mod.get_activation_tables = only_combined
    try:
        nc.compile()
    finally:
        hw_specs.get_activation_tables = orig
        bacc_mod.get_activation_tables = orig
    return nc


def _prep_inputs(x, ve, cos, sin, Wq, Wk, Wv, Wproj, Wgate, W):
    import ml_dtypes

    bf = ml_dtypes.bfloat16
    cosT = np.ascontiguousarray(cos[0, :, 0, :].T)  # (64, T)
    sinT = np.ascontiguousarray(sin[0, :, 0, :].T)
    cc = np.concatenate([cosT, cosT], axis=0)
    ss = np.concatenate([sinT, -sinT], axis=0)
    cs = np.ascontiguousarray(np.stack([cc, ss], axis=1)).astype(bf)  # (128,2,T)
    p = np.arange(128)[:, None]
    f = np.arange(128)[None, :]
    # -100/0 additive bias blocks, transposed for lhsT (bias.T @ I = bias)
    btri = np.ascontiguousarray(np.where(p <= f, 0.0, NEG).T).astype(bf)
    etri = np.ascontiguousarray(np.where(f <= p + (W % 128), 0.0, NEG).T).astype(bf)

    in_maps = []
    for core in range(8):
        b, g = core // NKV, core % NKV
        hs = slice(g * HPC * HD, (g + 1) * HPC * HD)
        ks = slice(g * HD, (g + 1) * HD)
        in_maps.append({
            "xT": np.ascontiguousarray(x[b].T).astype(bf),
            "wqT": np.ascontiguousarray(Wq[hs, :].T).astype(bf),
            "wkT": np.ascontiguousarray(Wk[ks, :].T).astype(bf),
            "wvT": np.ascontiguousarray(Wv[ks, :].T).astype(bf),
            "wpT": np.ascontiguousarray(Wproj[:, hs].T).astype(bf),
            "wg": np.ascontiguousarray(Wgate[g][:, None]).astype(bf),
            "cs": cs,
            "ve": np.ascontiguousarray(ve[b][:, ks]).astype(bf),
            "btri": btri,
            "etri": etri,
        })
    return in_maps


def _run(inputs, trace=False):
    from concourse.bass_utils import run_bass_kernel_spmd

    x = np.asarray(inputs["x"], dtype=np.float32)
    ve = np.asarray(inputs["ve"], dtype=np.float32)
    cos = np.asarray(inputs["cos"], dtype=np.float32)
    sin = np.asarray(inputs["sin"], dtype=np.float32)
    Wq = np.asarray(inputs["Wq"], dtype=np.float32)
    Wk = np.asarray(inputs["Wk"], dtype=np.float32)
    Wv = np.asarray(inputs["Wv"], dtype=np.float32)
    Wproj = np.asarray(inputs["Wproj"], dtype=np.float32)
    Wgate = np.asarray(inputs["Wgate"], dtype=np.float32)
    W = int(inputs["window_size"])

    if W not in _compiled:
        _compiled[W] = _build(W)
    nc = _compiled[W]

    in_maps = _prep_inputs(x, ve, cos, sin, Wq, Wk, Wv, Wproj, Wgate, W)
    res = run_bass_kernel_spmd(nc, in_maps, core_ids=list(range(8)), trace=trace)

    out = np.zeros((B, T, C), dtype=np.float32)
    for core in range(8):
        b = core // NKV
        out[b] += res.results[core]["outT"].T.astype(np.float32)
    return out, res


def kernel(**inputs):
    out, _ = _run(inputs, trace=False)
    return out


# revision 21
# speedup vs baseline: 1.0530x; 1.0530x over previous
"""Sliding-window causal self-attention (GQA + RoPE + QK-RMSNorm + ve-gate) on
8 Trainium2 NeuronCores.

Sharding: core c handles (batch b = c // 4, kv-head g = c % 4): data parallel
over batch x tensor parallel over the 4 KV head groups (4 query heads per
core). Each core computes its partial c_proj output; the all-reduce over the 4
head shards is a host-side sum.

Device design (per core):
  - x is fed transposed (xT: C x T) in bf16 so all projections contract over
    the partition axis at the full 1-col/cycle PE rate.
  - q, k are built transposed (qT/kT: head-dim x T); scores are computed
    TRANSPOSED (S^T: tk x tq) so softmax denominators come from a ones-matmul
    on the Tensor engine and P@V needs no transposes.
  - softmax skips max-subtraction: QK RMS-norm bounds |scores| <= 1.44*sqrt(128)
    so exp() cannot overflow. Masking multiplies the two triangular boundary
    blocks by {0,1} masks after exp.
  - k's rms-norm scale rides the per-partition `scale` operand of the Exp
    activation; q's rides the PSUM-evacuation multiply.
  - all matmuls run in bf16 (flat 1 col/cycle on the PE regardless of moving
    size); PSUM accumulation is fp32.
"""

import sys

sys.path.insert(0, "/opt/trn_rl_repo")

import numpy as np

B, T, C = 2, 2048, 2048
NH, NKV, HD = 16, 4, 128
GATE_CH = 12
HPC = NH // NKV          # q heads per core
TS = 512                 # token-slice width
NSL = T // TS            # 4 slices
NCK = C // 128           # 16 contraction chunks
TPS = TS // 128          # 4 token tiles per slice
NTT = T // 128           # 16 token tiles
EPS = 1e-6
NEG = -100.0

A_Q = 1.2 / np.sqrt(float(HD))   # rms-norm scale folded into q (incl 1/sqrt(HD))
A_K = 1.2                        # rms-norm scale folded into exp() scale arg
S_Q = float(1.0 / (HD * A_Q * A_Q))
B_Q = float(EPS / (A_Q * A_Q))
S_K = float(1.0 / (HD * A_K * A_K))
B_K = float(EPS / (A_K * A_K))

_compiled = {}


def _ktiles(m4, W):
    """k-tiles overlapping q-slice m4 with their valid tq-column extents.

    Returns list of (n, f0, f1, causal_block_col, edge_block_col); columns are
    relative to the slice (0..TS). First entry covers [0, TS) fully (it opens
    the PSUM accumulation group).
    """
    assert W % 128 == 0 and W >= 384
    out = []
    for n in range(0, TPS * m4 + TPS):
        f0 = max(0, 128 * n - TS * m4)
        f1 = min(TS, 128 * n + W + 128 - TS * m4)
        if f1 <= f0:
            continue
        causal = 128 * n >= TS * m4            # diagonal staircase inside tile
        edge = (128 * n + W + 128 - TS * m4) <= TS  # window lower edge inside
        cb = f0 if causal else None
        eb = (f1 - 128) if edge else None
        out.append((n, f0, f1, cb, eb))
    full = [e for e in out if e[1] == 0 and e[2] == TS]
    assert full, "need one full-extent tile to open the PSUM group"
    first = full[0]
    rest = [e for e in out if e[0] != first[0]]
    return [first] + rest


def _build(W):
    import concourse.bass as bass
    import concourse.tile as tile
    from concourse import bacc, mybir
    from concourse.masks import make_identity
    from contextlib import ExitStack

    f32 = mybir.dt.float32
    f32r = mybir.dt.float32r
    bf16 = mybir.dt.bfloat16
    AF = mybir.ActivationFunctionType
    OP = mybir.AluOpType

    nc = bacc.Bacc(None, target_bir_lowering=False)

    xT = nc.dram_tensor("xT", [C, T], bf16, kind="ExternalInput")
    wq = nc.dram_tensor("wqT", [C, HPC * HD], bf16, kind="ExternalInput")
    wk = nc.dram_tensor("wkT", [C, HD], bf16, kind="ExternalInput")
    wv = nc.dram_tensor("wvT", [C, HD], bf16, kind="ExternalInput")
    wp = nc.dram_tensor("wpT", [HPC * HD, C], bf16, kind="ExternalInput")
    wgd = nc.dram_tensor("wg", [GATE_CH, 1], bf16, kind="ExternalInput")
    csd = nc.dram_tensor("cs", [HD, 2, T], bf16, kind="ExternalInput")
    ved = nc.dram_tensor("ve", [T, HD], bf16, kind="ExternalInput")
    btrid = nc.dram_tensor("btri", [128, 128], bf16, kind="ExternalInput")
    etrid = nc.dram_tensor("etri", [128, 128], bf16, kind="ExternalInput")
    outT = nc.dram_tensor("outT", [C, T], bf16, kind="ExternalOutput")

    with tile.TileContext(nc) as tc, ExitStack() as ctx:
        res = ctx.enter_context(tc.tile_pool(name="res", bufs=1))
        xc_p = ctx.enter_context(tc.tile_pool(name="xc", bufs=2))
        tab_p = ctx.enter_context(tc.tile_pool(name="tab", bufs=2))
        work_p = ctx.enter_context(tc.tile_pool(name="work", bufs=2))
        sq_p = ctx.enter_context(tc.tile_pool(name="sq", bufs=3))
        bc_p = ctx.enter_context(tc.tile_pool(name="bc", bufs=2))
        qt_p = ctx.enter_context(tc.tile_pool(name="qt", bufs=2))
        es_p = ctx.enter_context(tc.tile_pool(name="es", bufs=4))
        yt_p = ctx.enter_context(tc.tile_pool(name="yt", bufs=1))
        ot_p = ctx.enter_context(tc.tile_pool(name="ot", bufs=3))
        row_p = ctx.enter_context(tc.tile_pool(name="rows", bufs=1))

        ps_qkv = ctx.enter_context(tc.tile_pool(name="ps_qkv", bufs=2, space="PSUM"))
        ps_s = ctx.enter_context(tc.tile_pool(name="ps_s", bufs=3, space="PSUM"))
        ps_row = ctx.enter_context(tc.tile_pool(name="ps_row", bufs=3, space="PSUM"))
        dram_p = ctx.enter_context(tc.tile_pool(name="dram", bufs=2, space="DRAM"))

        # resident tensors; weight loads split per chunk so the first QKV
        # matmuls can start as soon as their chunk lands (startup latency).
        wq_sb = res.tile([128, NCK, HPC * HD], bf16)
        wk_sb = res.tile([128, NCK, HD], bf16)
        wv_sb = res.tile([128, NCK, HD], bf16)
        wp_sb = res.tile([128, HPC, C], bf16)   # loaded later, before cproj(0)
        wg_sb = res.tile([GATE_CH, 1], bf16)
        nc.sync.dma_start(out=wg_sb, in_=wgd[:, :])
        btri_sb = res.tile([128, 128], bf16)   # -100/0 bias, transposed (lhsT)
        nc.sync.dma_start(out=btri_sb, in_=btrid[:, :])
        etri_sb = res.tile([128, 128], bf16)
        nc.sync.dma_start(out=etri_sb, in_=etrid[:, :])
        ident = res.tile([128, 128], f32)
        make_identity(nc, ident)
        ident_bf = res.tile([128, 128], bf16)
        make_identity(nc, ident_bf)
        ones_sb = res.tile([128, 1], bf16)
        nc.vector.memset(ones_sb, 1.0)
        bq_sb = res.tile([1, 1], f32)
        nc.vector.memset(bq_sb, B_Q)
        bk_sb = res.tile([128, 1], f32)
        nc.vector.memset(bk_sb, B_K)
        kT_sb = res.tile([128, T], bf16)        # rotated k, head-dim on partitions
        vn_sb = res.tile([128, NTT, HD], bf16)  # v natural, token tiles on partitions
        rnk_sb = res.tile([128, NTT], f32)      # per-k-tile rms-norm columns

        def rope_inplace(dst, cc_sl, ss_sl):
            """dst (128, TS) bf16 holding pre-rotation values. In-place RoPE."""
            qsw = work_p.tile([128, TS], bf16, tag="qsw")
            nc.sync.dma_start(out=qsw[0:64, :], in_=dst[64:128, :])
            nc.sync.dma_start(out=qsw[64:128, :], in_=dst[0:64, :])
            tmp = work_p.tile([128, TS], bf16, tag="tmp")
            nc.vector.tensor_mul(tmp, qsw, ss_sl)
            nc.vector.tensor_mul(dst, dst, cc_sl)
            nc.vector.tensor_add(dst, dst, tmp)

        for m4 in range(NSL):
            t0 = m4 * TS
            # ---- stream x slice + tables ----
            xca = xc_p.tile([128, NCK, TS], bf16, tag="xc")
            xsrc = xT[:, t0:t0 + TS].rearrange("(ck p) t -> p ck t", p=128)
            if m4 == 0:
                # chunk 0 lands first so the gate/k matmuls start early
                nc.sync.dma_start(out=xca[:, 0:1, :], in_=xsrc[:, 0:1, :])
                nc.sync.dma_start(out=wk_sb[:, :, :],
                                  in_=wk.rearrange("(ck p) h -> p ck h", p=128))
                nc.sync.dma_start(out=xca[:, 1:8, :], in_=xsrc[:, 1:8, :])
                nc.sync.dma_start(out=xca[:, 8:NCK, :], in_=xsrc[:, 8:NCK, :])
            else:
                nc.sync.dma_start(out=xca[:, 0:8, :], in_=xsrc[:, 0:8, :])
                nc.sync.dma_start(out=xca[:, 8:NCK, :], in_=xsrc[:, 8:NCK, :])
            xc = [xca[:, c, :] for c in range(NCK)]
            cs_sl = tab_p.tile([128, 2, TS], bf16, tag="cs")
            nc.sync.dma_start(out=cs_sl, in_=csd[:, :, t0:t0 + TS])
            cc_sl = cs_sl[:, 0, :]
            ss_sl = cs_sl[:, 1, :]
            ve_sl = tab_p.tile([128, TPS, HD], bf16, tag="ve")
            nc.sync.dma_start(
                out=ve_sl, in_=ved[t0:t0 + TS, :].rearrange("(tt p) h -> p tt h", p=128)
            )

            # ---- gate columns: 3*sigmoid(x[:, :12] @ wg) ----
            ps_g = ps_row.tile([1, TS], f32, tag="rows")
            nc.tensor.matmul(ps_g, wg_sb, xc[0][0:GATE_CH, :], start=True, stop=True)
            g_row = row_p.tile([1, TS], f32, tag="grow")
            nc.scalar.activation(g_row, ps_g, AF.Exp, scale=-1.0)
            nc.vector.tensor_scalar(out=g_row, in0=g_row, scalar1=1.0, scalar2=None,
                                    op0=OP.add)
            nc.vector.reciprocal(g_row, g_row)
            g_dr = dram_p.tile([TS], f32, tag="gdr")
            nc.sync.dma_start(out=g_dr, in_=g_row)
            gate_c = row_p.tile([128, TPS], f32, tag="gate")
            nc.sync.dma_start(
                out=gate_c,
                in_=bass.AP(tensor=g_dr.tensor, offset=g_dr.offset,
                            ap=[[1, 128], [128, TPS]]),
            )

            # ---- k projection + rms-norm cols + rope ----
            ps_k = ps_qkv.tile([128, TS], f32, tag="qkv")
            for c in range(NCK):
                nc.tensor.matmul(ps_k, wk_sb[:, c, :], xc[c],
                                 start=(c == 0), stop=(c == NCK - 1))
            sq_k = sq_p.tile([128, TS], bf16, tag="sq")
            nc.scalar.activation(sq_k, ps_k, AF.Square)
            ps_rk = ps_row.tile([1, TS], f32, tag="rows")
            nc.tensor.matmul(ps_rk, ones_sb, sq_k, start=True, stop=True)
            srk = row_p.tile([1, TS], f32, tag="srk")
            nc.scalar.activation(srk, ps_rk, AF.Ln, bias=bk_sb[0:1], scale=S_K)
            nc.scalar.activation(srk, srk, AF.Exp, scale=-0.5)
            k_dr = dram_p.tile([TS], f32, tag="kdr")
            nc.sync.dma_start(out=k_dr, in_=srk)
            nc.sync.dma_start(
                out=rnk_sb[:, m4 * TPS:(m4 + 1) * TPS],
                in_=bass.AP(tensor=k_dr.tensor, offset=k_dr.offset,
                            ap=[[1, 128], [128, TPS]]),
            )
            k_sl = kT_sb[:, t0:t0 + TS]
            nc.vector.tensor_copy(k_sl, ps_k)
            rope_inplace(k_sl, cc_sl, ss_sl)

            # ---- v projection + transpose to natural + gate-add ----
            if m4 == 0:
                nc.sync.dma_start(out=wv_sb[:, :, :],
                                  in_=wv.rearrange("(ck p) h -> p ck h", p=128))
            ps_v = ps_qkv.tile([128, TS], f32, tag="qkv")
            for c in range(NCK):
                nc.tensor.matmul(ps_v, wv_sb[:, c, :], xc[c],
                                 start=(c == 0), stop=(c == NCK - 1))
            vT_s = work_p.tile([128, TS], f32, tag="vts")
            nc.vector.tensor_copy(vT_s, ps_v)
            for tt in range(TPS):
                ps_t = ps_s.tile([128, TS], f32, tag="s")
                nc.tensor.transpose(ps_t[:, 0:128], vT_s[:, tt * 128:(tt + 1) * 128],
                                    ident)
                gtmp = work_p.tile([128, HD], bf16, tag="gtmp")
                nc.vector.tensor_scalar(out=gtmp, in0=ve_sl[:, tt, :],
                                        scalar1=gate_c[:, tt:tt + 1], scalar2=3.0,
                                        op0=OP.mult, op1=OP.mult)
                nc.vector.tensor_add(vn_sb[:, m4 * TPS + tt, :], ps_t[:, 0:128], gtmp)

            # ---- q projections (4 heads) + rms-norm + rope ----
            if m4 == 0:
                wqsrc = wq.rearrange("(ck p) h -> p ck h", p=128)
                for cg in range(0, NCK, 4):
                    nc.sync.dma_start(out=wq_sb[:, cg:cg + 4, :],
                                      in_=wqsrc[:, cg:cg + 4, :])
            qts = []
            for h in range(HPC):
                ps_q = ps_qkv.tile([128, TS], f32, tag="qkv")
                for c in range(NCK):
                    nc.tensor.matmul(ps_q, wq_sb[:, c, h * HD:(h + 1) * HD], xc[c],
                                     start=(c == 0), stop=(c == NCK - 1))
                sq_q = sq_p.tile([128, TS], bf16, tag="sq")
                nc.scalar.activation(sq_q, ps_q, AF.Square)
                ps_r = ps_row.tile([1, TS], f32, tag="rows")
                nc.tensor.matmul(ps_r, ones_sb, sq_q, start=True, stop=True)
                srow = row_p.tile([1, TS], f32, tag="srow")
                nc.scalar.activation(srow, ps_r, AF.Ln, bias=bq_sb, scale=S_Q)
                nc.scalar.activation(srow, srow, AF.Exp, scale=-0.5)
                rbc = bc_p.tile([128, TS], f32, tag="bc")
                nc.gpsimd.partition_broadcast(rbc, srow)
                qt = qt_p.tile([128, TS], bf16, tag=f"qt{h}")
                nc.vector.tensor_mul(qt, ps_q, rbc)
                rope_inplace(qt, cc_sl, ss_sl)
                qts.append(qt)

            # ---- attention (scores transposed: tk on partitions, tq free) ----
            tiles = _ktiles(m4, W)
            last = len(tiles) - 1
            yts = []
            for h in range(HPC):
                ps_out = ps_row.tile([128, TS], f32, tag="rows")
                ps_sum = ps_row.tile([1, TS], f32, tag="rows")
                for idx, (n, f0, f1, cb, eb) in enumerate(tiles):
                    pss = ps_s.tile([128, TS], f32, tag="s")
                    nc.tensor.matmul(pss[:, f0:f1], kT_sb[:, n * 128:(n + 1) * 128],
                                     qts[h][:, f0:f1], start=True, stop=True)
                    es = es_p.tile([128, TS], bf16, tag="es")
                    nc.scalar.activation(es[:, f0:f1], pss[:, f0:f1], AF.Exp,
                                         scale=rnk_sb[:, n:n + 1])
                    if cb is not None:
                        nc.gpsimd.tensor_mul(es[:, cb:cb + 128],
                                             es[:, cb:cb + 128], btri_sb)
                    if eb is not None:
                        nc.gpsimd.tensor_mul(es[:, eb:eb + 128],
                                             es[:, eb:eb + 128], etri_sb)
                    nc.tensor.matmul(ps_sum[:, f0:f1], ones_sb, es[:, f0:f1],
                                     start=(idx == 0), stop=(idx == last))
                    nc.tensor.matmul(ps_out[:, f0:f1], vn_sb[:, n, :], es[:, f0:f1],
                                     start=(idx == 0), stop=(idx == last))
                rsum = row_p.tile([1, TS], f32, tag="rsum")
                nc.vector.reciprocal(rsum, ps_sum)
                sbc = bc_p.tile([128, TS], f32, tag="bc")
                nc.gpsimd.partition_broadcast(sbc, rsum)
                yt = yt_p.tile([128, TS], bf16, tag=f"yt{h}")
                nc.vector.tensor_mul(yt, ps_out, sbc)
                yts.append(yt)

            # ---- c_proj partial: outT[co, t] = sum_h wpT[h].T @ yT[h] ----
            if m4 == 0:
                nc.sync.dma_start(out=wp_sb[:, :, :],
                                  in_=wp.rearrange("(h p) c -> p h c", p=128))
            for co4 in range(0, NTT, 4):
                ot = ot_p.tile([128, 4, TS], bf16, tag="ot")
                for ci in range(4):
                    co = co4 + ci
                    ps_p = ps_s.tile([128, TS], f32, tag="s")
                    for h in range(HPC):
                        nc.tensor.matmul(ps_p, wp_sb[:, h, co * 128:(co + 1) * 128],
                                         yts[h], start=(h == 0), stop=(h == HPC - 1))
                    if ci % 2 == 0:
                        nc.vector.tensor_copy(ot[:, ci, :], ps_p)
                    else:
                        nc.scalar.copy(ot[:, ci, :], ps_p)
                nc.sync.dma_start(
                    out=outT[co4 * 128:(co4 + 4) * 128,
                             t0:t0 + TS].rearrange("(ck p) t -> p ck t", p=128),
                    in_=ot)

    # Restrict the activation-table picker to the one set containing every
    # ACT function we use (exp, ln, square, copy, identity): without this the
    # greedy picker alternates exp_and_others <-> natural_log, inserting a
    # ~1.3us table load per switch. Set ids are positions in act_info.json's
    # list, so unwanted sets are emptied rather than removed.
    import concourse.hw_specs as hw_specs
    import concourse.bacc as bacc_mod

    orig = hw_specs.get_activation_tables

    def only_combined(arch):
        t = orig(arch)
        return {k: (v if k == "natural_log_exp_and_others" else set())
                for k, v in t.items()}

    hw_specs.get_activation_tables = only_combined
    bacc_mod.get_activation_tables = only_combined
    try:
        nc.compile()
    finally:
        hw_specs.get_activation_tables = orig
        bacc_mod.get_activation_tables = orig
    return nc


def _prep_inputs(x, ve, cos, sin, Wq, Wk, Wv, Wproj, Wgate, W):
    import ml_dtypes

    bf = ml_dtypes.bfloat16
    cosT = np.ascontiguousarray(cos[0, :, 0, :].T)  # (64, T)
    sinT = np.ascontiguousarray(sin[0, :, 0, :].T)
    cc = np.concatenate([cosT, cosT], axis=0)
    ss = np.concatenate([sinT, -sinT], axis=0)
    cs = np.ascontiguousarray(np.stack([cc, ss], axis=1)).astype(bf)  # (128,2,T)
    p = np.arange(128)[:, None]
    f = np.arange(128)[None, :]
    btri = (p <= f).astype(bf)
    etri = (f <= p + (W % 128)).astype(bf)

    in_maps = []
    for core in range(8):
        b, g = core // NKV, core % NKV
        hs = slice(g * HPC * HD, (g + 1) * HPC * HD)
        ks = slice(g * HD, (g + 1) * HD)
        in_maps.append({
            "xT": np.ascontiguousarray(x[b].T).astype(bf),
            "wqT": np.ascontiguousarray(Wq[hs, :].T).astype(bf),
            "wkT": np.ascontiguousarray(Wk[ks, :].T).astype(bf),
            "wvT": np.ascontiguousarray(Wv[ks, :].T).astype(bf),
            "wpT": np.ascontiguousarray(Wproj[:, hs].T).astype(bf),
            "wg": np.ascontiguousarray(Wgate[g][:, None]).astype(bf),
            "cs": cs,
            "ve": np.ascontiguousarray(ve[b][:, ks]).astype(bf),
            "btri": btri,
            "etri": etri,
        })
    return in_maps


def _run(inputs, trace=False):
    from concourse.bass_utils import run_bass_kernel_spmd

    x = np.asarray(inputs["x"], dtype=np.float32)
    ve = np.asarray(inputs["ve"], dtype=np.float32)
    cos = np.asarray(inputs["cos"], dtype=np.float32)
    sin = np.asarray(inputs["sin"], dtype=np.float32)
    Wq = np.asarray(inputs["Wq"], dtype=np.float32)
    Wk = np.asarray(inputs["Wk"], dtype=np.float32)
    Wv = np.asarray(inputs["Wv"], dtype=np.float32)
    Wproj = np.asarray(inputs["Wproj"], dtype=np.float32)
    Wgate = np.asarray(inputs["Wgate"], dtype=np.float32)
    W = int(inputs["window_size"])

    if W not in _compiled:
        _compiled[W] = _build(W)
    nc = _compiled[W]

    in_maps = _prep_inputs(x, ve, cos, sin, Wq, Wk, Wv, Wproj, Wgate, W)
    res = run_bass_kernel_spmd(nc, in_maps, core_ids=list(range(8)), trace=trace)

    out = np.zeros((B, T, C), dtype=np.float32)
    for core in range(8):
        b = core // NKV
        out[b] += res.results[core]["outT"].T.astype(np.float32)
    return out, res


def kernel(**inputs):
    out, _ = _run(inputs, trace=False)
    return out


# revision 22
# speedup vs baseline: 1.0547x; 1.0016x over previous
"""Sliding-window causal self-attention (GQA + RoPE + QK-RMSNorm + ve-gate) on
8 Trainium2 NeuronCores.

Sharding: core c handles (batch b = c // 4, kv-head g = c % 4): data parallel
over batch x tensor parallel over the 4 KV head groups (4 query heads per
core). Each core computes its partial c_proj output; the all-reduce over the 4
head shards is a host-side sum.

Device design (per core):
  - x is fed transposed (xT: C x T) in bf16 so all projections contract over
    the partition axis at the full 1-col/cycle PE rate.
  - q, k are built transposed (qT/kT: head-dim x T); scores are computed
    TRANSPOSED (S^T: tk x tq) so softmax denominators come from a ones-matmul
    on the Tensor engine and P@V needs no transposes.
  - softmax skips max-subtraction: QK RMS-norm bounds |scores| <= 1.44*sqrt(128)
    so exp() cannot overflow. Masking multiplies the two triangular boundary
    blocks by {0,1} masks after exp.
  - k's rms-norm scale rides the per-partition `scale` operand of the Exp
    activation; q's rides the PSUM-evacuation multiply.
  - all matmuls run in bf16 (flat 1 col/cycle on the PE regardless of moving
    size); PSUM accumulation is fp32.
"""

import sys

sys.path.insert(0, "/opt/trn_rl_repo")

import numpy as np

B, T, C = 2, 2048, 2048
NH, NKV, HD = 16, 4, 128
GATE_CH = 12
HPC = NH // NKV          # q heads per core
TS = 512                 # token-slice width
NSL = T // TS            # 4 slices
NCK = C // 128           # 16 contraction chunks
TPS = TS // 128          # 4 token tiles per slice
NTT = T // 128           # 16 token tiles
EPS = 1e-6
NEG = -100.0

A_Q = 1.2 / np.sqrt(float(HD))   # rms-norm scale folded into q (incl 1/sqrt(HD))
A_K = 1.2                        # rms-norm scale folded into exp() scale arg
S_Q = float(1.0 / (HD * A_Q * A_Q))
B_Q = float(EPS / (A_Q * A_Q))
S_K = float(1.0 / (HD * A_K * A_K))
B_K = float(EPS / (A_K * A_K))

_compiled = {}


def _ktiles(m4, W):
    """k-tiles overlapping q-slice m4 with their valid tq-column extents.

    Returns list of (n, f0, f1, causal_block_col, edge_block_col); columns are
    relative to the slice (0..TS). First entry covers [0, TS) fully (it opens
    the PSUM accumulation group).
    """
    assert W % 128 == 0 and W >= 384
    out = []
    for n in range(0, TPS * m4 + TPS):
        f0 = max(0, 128 * n - TS * m4)
        f1 = min(TS, 128 * n + W + 128 - TS * m4)
        if f1 <= f0:
            continue
        causal = 128 * n >= TS * m4            # diagonal staircase inside tile
        edge = (128 * n + W + 128 - TS * m4) <= TS  # window lower edge inside
        cb = f0 if causal else None
        eb = (f1 - 128) if edge else None
        out.append((n, f0, f1, cb, eb))
    full = [e for e in out if e[1] == 0 and e[2] == TS]
    assert full, "need one full-extent tile to open the PSUM group"
    first = full[0]
    rest = [e for e in out if e[0] != first[0]]
    return [first] + rest


def _build(W):
    import concourse.bass as bass
    import concourse.tile as tile
    from concourse import bacc, mybir
    from concourse.masks import make_identity
    from contextlib import ExitStack

    f32 = mybir.dt.float32
    f32r = mybir.dt.float32r
    bf16 = mybir.dt.bfloat16
    AF = mybir.ActivationFunctionType
    OP = mybir.AluOpType

    nc = bacc.Bacc(None, target_bir_lowering=False)

    xT = nc.dram_tensor("xT", [C, T], bf16, kind="ExternalInput")
    wq = nc.dram_tensor("wqT", [C, HPC * HD], bf16, kind="ExternalInput")
    wk = nc.dram_tensor("wkT", [C, HD], bf16, kind="ExternalInput")
    wv = nc.dram_tensor("wvT", [C, HD], bf16, kind="ExternalInput")
    wp = nc.dram_tensor("wpT", [HPC * HD, C], bf16, kind="ExternalInput")
    wgd = nc.dram_tensor("wg", [GATE_CH, 1], bf16, kind="ExternalInput")
    csd = nc.dram_tensor("cs", [HD, 2, T], bf16, kind="ExternalInput")
    ved = nc.dram_tensor("ve", [T, HD], bf16, kind="ExternalInput")
    btrid = nc.dram_tensor("btri", [128, 128], bf16, kind="ExternalInput")
    etrid = nc.dram_tensor("etri", [128, 128], bf16, kind="ExternalInput")
    outT = nc.dram_tensor("outT", [C, T], bf16, kind="ExternalOutput")

    with tile.TileContext(nc) as tc, ExitStack() as ctx:
        res = ctx.enter_context(tc.tile_pool(name="res", bufs=1))
        xc_p = ctx.enter_context(tc.tile_pool(name="xc", bufs=2))
        tab_p = ctx.enter_context(tc.tile_pool(name="tab", bufs=2))
        work_p = ctx.enter_context(tc.tile_pool(name="work", bufs=3))
        sq_p = ctx.enter_context(tc.tile_pool(name="sq", bufs=4))
        bc_p = ctx.enter_context(tc.tile_pool(name="bc", bufs=3))
        qt_p = ctx.enter_context(tc.tile_pool(name="qt", bufs=2))
        es_p = ctx.enter_context(tc.tile_pool(name="es", bufs=6))
        yt_p = ctx.enter_context(tc.tile_pool(name="yt", bufs=1))
        ot_p = ctx.enter_context(tc.tile_pool(name="ot", bufs=2))
        row_p = ctx.enter_context(tc.tile_pool(name="rows", bufs=2))

        ps_qkv = ctx.enter_context(tc.tile_pool(name="ps_qkv", bufs=2, space="PSUM"))
        ps_s = ctx.enter_context(tc.tile_pool(name="ps_s", bufs=3, space="PSUM"))
        ps_row = ctx.enter_context(tc.tile_pool(name="ps_row", bufs=3, space="PSUM"))
        dram_p = ctx.enter_context(tc.tile_pool(name="dram", bufs=2, space="DRAM"))

        # resident tensors; weight loads split per chunk so the first QKV
        # matmuls can start as soon as their chunk lands (startup latency).
        wq_sb = res.tile([128, NCK, HPC * HD], bf16)
        wk_sb = res.tile([128, NCK, HD], bf16)
        wv_sb = res.tile([128, NCK, HD], bf16)
        wp_sb = res.tile([128, HPC, C], bf16)   # loaded later, before cproj(0)
        wg_sb = res.tile([GATE_CH, 1], bf16)
        nc.sync.dma_start(out=wg_sb, in_=wgd[:, :])
        btri_sb = res.tile([128, 128], bf16)   # -100/0 bias, transposed (lhsT)
        nc.sync.dma_start(out=btri_sb, in_=btrid[:, :])
        etri_sb = res.tile([128, 128], bf16)
        nc.sync.dma_start(out=etri_sb, in_=etrid[:, :])
        ident = res.tile([128, 128], f32)
        make_identity(nc, ident)
        ident_bf = res.tile([128, 128], bf16)
        make_identity(nc, ident_bf)
        ones_sb = res.tile([128, 1], bf16)
        nc.vector.memset(ones_sb, 1.0)
        bq_sb = res.tile([1, 1], f32)
        nc.vector.memset(bq_sb, B_Q)
        bk_sb = res.tile([128, 1], f32)
        nc.vector.memset(bk_sb, B_K)
        kT_sb = res.tile([128, T], bf16)        # rotated k, head-dim on partitions
        vn_sb = res.tile([128, NTT, HD], bf16)  # v natural, token tiles on partitions
        rnk_sb = res.tile([128, NTT], f32)      # per-k-tile rms-norm columns

        def rope_inplace(dst, cc_sl, ss_sl):
            """dst (128, TS) bf16 holding pre-rotation values. In-place RoPE."""
            qsw = work_p.tile([128, TS], bf16, tag="qsw")
            nc.sync.dma_start(out=qsw[0:64, :], in_=dst[64:128, :])
            nc.sync.dma_start(out=qsw[64:128, :], in_=dst[0:64, :])
            tmp = work_p.tile([128, TS], bf16, tag="tmp")
            nc.vector.tensor_mul(tmp, qsw, ss_sl)
            nc.vector.tensor_mul(dst, dst, cc_sl)
            nc.vector.tensor_add(dst, dst, tmp)

        for m4 in range(NSL):
            t0 = m4 * TS
            # ---- stream x slice + tables ----
            xca = xc_p.tile([128, NCK, TS], bf16, tag="xc")
            xsrc = xT[:, t0:t0 + TS].rearrange("(ck p) t -> p ck t", p=128)
            if m4 == 0:
                # chunk 0 lands first so the gate/k matmuls start early
                nc.sync.dma_start(out=xca[:, 0:1, :], in_=xsrc[:, 0:1, :])
                nc.sync.dma_start(out=wk_sb[:, :, :],
                                  in_=wk.rearrange("(ck p) h -> p ck h", p=128))
                nc.sync.dma_start(out=xca[:, 1:8, :], in_=xsrc[:, 1:8, :])
                nc.sync.dma_start(out=xca[:, 8:NCK, :], in_=xsrc[:, 8:NCK, :])
            else:
                nc.sync.dma_start(out=xca[:, 0:8, :], in_=xsrc[:, 0:8, :])
                nc.sync.dma_start(out=xca[:, 8:NCK, :], in_=xsrc[:, 8:NCK, :])
            xc = [xca[:, c, :] for c in range(NCK)]
            cs_sl = tab_p.tile([128, 2, TS], bf16, tag="cs")
            nc.sync.dma_start(out=cs_sl, in_=csd[:, :, t0:t0 + TS])
            cc_sl = cs_sl[:, 0, :]
            ss_sl = cs_sl[:, 1, :]
            ve_sl = tab_p.tile([128, TPS, HD], bf16, tag="ve")
            nc.sync.dma_start(
                out=ve_sl, in_=ved[t0:t0 + TS, :].rearrange("(tt p) h -> p tt h", p=128)
            )

            # ---- gate columns: 3*sigmoid(x[:, :12] @ wg) ----
            ps_g = ps_row.tile([1, TS], f32, tag="rows")
            nc.tensor.matmul(ps_g, wg_sb, xc[0][0:GATE_CH, :], start=True, stop=True)
            g_row = row_p.tile([1, TS], f32, tag="grow")
            nc.scalar.activation(g_row, ps_g, AF.Exp, scale=-1.0)
            nc.vector.tensor_scalar(out=g_row, in0=g_row, scalar1=1.0, scalar2=None,
                                    op0=OP.add)
            nc.vector.reciprocal(g_row, g_row)
            g_dr = dram_p.tile([TS], f32, tag="gdr")
            nc.sync.dma_start(out=g_dr, in_=g_row)
            gate_c = row_p.tile([128, TPS], f32, tag="gate")
            nc.sync.dma_start(
                out=gate_c,
                in_=bass.AP(tensor=g_dr.tensor, offset=g_dr.offset,
                            ap=[[1, 128], [128, TPS]]),
            )

            # ---- k projection + rms-norm cols + rope ----
            ps_k = ps_qkv.tile([128, TS], f32, tag="qkv")
            for c in range(NCK):
                nc.tensor.matmul(ps_k, wk_sb[:, c, :], xc[c],
                                 start=(c == 0), stop=(c == NCK - 1))
            sq_k = sq_p.tile([128, TS], bf16, tag="sq")
            nc.scalar.activation(sq_k, ps_k, AF.Square)
            ps_rk = ps_row.tile([1, TS], f32, tag="rows")
            nc.tensor.matmul(ps_rk, ones_sb, sq_k, start=True, stop=True)
            srk = row_p.tile([1, TS], f32, tag="srk")
            nc.scalar.activation(srk, ps_rk, AF.Ln, bias=bk_sb[0:1], scale=S_K)
            nc.scalar.activation(srk, srk, AF.Exp, scale=-0.5)
            k_dr = dram_p.tile([TS], f32, tag="kdr")
            nc.sync.dma_start(out=k_dr, in_=srk)
            nc.sync.dma_start(
                out=rnk_sb[:, m4 * TPS:(m4 + 1) * TPS],
                in_=bass.AP(tensor=k_dr.tensor, offset=k_dr.offset,
                            ap=[[1, 128], [128, TPS]]),
            )
            k_sl = kT_sb[:, t0:t0 + TS]
            nc.vector.tensor_copy(k_sl, ps_k)
            rope_inplace(k_sl, cc_sl, ss_sl)

            # ---- v projection + transpose to natural + gate-add ----
            if m4 == 0:
                nc.sync.dma_start(out=wv_sb[:, :, :],
                                  in_=wv.rearrange("(ck p) h -> p ck h", p=128))
            ps_v = ps_qkv.tile([128, TS], f32, tag="qkv")
            for c in range(NCK):
                nc.tensor.matmul(ps_v, wv_sb[:, c, :], xc[c],
                                 start=(c == 0), stop=(c == NCK - 1))
            vT_s = work_p.tile([128, TS], f32, tag="vts")
            nc.vector.tensor_copy(vT_s, ps_v)
            for tt in range(TPS):
                ps_t = ps_s.tile([128, TS], f32, tag="s")
                nc.tensor.transpose(ps_t[:, 0:128], vT_s[:, tt * 128:(tt + 1) * 128],
                                    ident)
                gtmp = work_p.tile([128, HD], bf16, tag="gtmp")
                nc.vector.tensor_scalar(out=gtmp, in0=ve_sl[:, tt, :],
                                        scalar1=gate_c[:, tt:tt + 1], scalar2=3.0,
                                        op0=OP.mult, op1=OP.mult)
                nc.vector.tensor_add(vn_sb[:, m4 * TPS + tt, :], ps_t[:, 0:128], gtmp)

            # ---- q projections (4 heads) + rms-norm + rope ----
            if m4 == 0:
                wqsrc = wq.rearrange("(ck p) h -> p ck h", p=128)
                for cg in range(0, NCK, 4):
                    nc.sync.dma_start(out=wq_sb[:, cg:cg + 4, :],
                                      in_=wqsrc[:, cg:cg + 4, :])
            qts = []
            for h in range(HPC):
                ps_q = ps_qkv.tile([128, TS], f32, tag="qkv")
                for c in range(NCK):
                    nc.tensor.matmul(ps_q, wq_sb[:, c, h * HD:(h + 1) * HD], xc[c],
                                     start=(c == 0), stop=(c == NCK - 1))
                sq_q = sq_p.tile([128, TS], bf16, tag="sq")
                nc.scalar.activation(sq_q, ps_q, AF.Square)
                ps_r = ps_row.tile([1, TS], f32, tag="rows")
                nc.tensor.matmul(ps_r, ones_sb, sq_q, start=True, stop=True)
                srow = row_p.tile([1, TS], f32, tag="srow")
                nc.scalar.activation(srow, ps_r, AF.Ln, bias=bq_sb, scale=S_Q)
                nc.scalar.activation(srow, srow, AF.Exp, scale=-0.5)
                rbc = bc_p.tile([128, TS], f32, tag="bc")
                nc.gpsimd.partition_broadcast(rbc, srow)
                qt = qt_p.tile([128, TS], bf16, tag=f"qt{h}")
                nc.vector.tensor_mul(qt, ps_q, rbc)
                rope_inplace(qt, cc_sl, ss_sl)
                qts.append(qt)

            # ---- attention (scores transposed: tk on partitions, tq free) ----
            tiles = _ktiles(m4, W)
            last = len(tiles) - 1
            yts = []
            for h in range(HPC):
                ps_out = ps_row.tile([128, TS], f32, tag="rows")
                ps_sum = ps_row.tile([1, TS], f32, tag="rows")
                for idx, (n, f0, f1, cb, eb) in enumerate(tiles):
                    pss = ps_s.tile([128, TS], f32, tag="s")
                    nc.tensor.matmul(pss[:, f0:f1], kT_sb[:, n * 128:(n + 1) * 128],
                                     qts[h][:, f0:f1], start=True, stop=True)
                    es = es_p.tile([128, TS], bf16, tag="es")
                    nc.scalar.activation(es[:, f0:f1], pss[:, f0:f1], AF.Exp,
                                         scale=rnk_sb[:, n:n + 1])
                    if cb is not None:
                        nc.gpsimd.tensor_mul(es[:, cb:cb + 128],
                                             es[:, cb:cb + 128], btri_sb)
                    if eb is not None:
                        nc.gpsimd.tensor_mul(es[:, eb:eb + 128],
                                             es[:, eb:eb + 128], etri_sb)
                    nc.tensor.matmul(ps_sum[:, f0:f1], ones_sb, es[:, f0:f1],
                                     start=(idx == 0), stop=(idx == last))
                    nc.tensor.matmul(ps_out[:, f0:f1], vn_sb[:, n, :], es[:, f0:f1],
                                     start=(idx == 0), stop=(idx == last))
                rsum = row_p.tile([1, TS], f32, tag="rsum")
                nc.vector.reciprocal(rsum, ps_sum)
                sbc = bc_p.tile([128, TS], f32, tag="bc")
                nc.gpsimd.partition_broadcast(sbc, rsum)
                yt = yt_p.tile([128, TS], bf16, tag=f"yt{h}")
                nc.vector.tensor_mul(yt, ps_out, sbc)
                yts.append(yt)

            # ---- c_proj partial: outT[co, t] = sum_h wpT[h].T @ yT[h] ----
            if m4 == 0:
                nc.sync.dma_start(out=wp_sb[:, :, :],
                                  in_=wp.rearrange("(h p) c -> p h c", p=128))
            for co4 in range(0, NTT, 4):
                ot = ot_p.tile([128, 4, TS], bf16, tag="ot")
                for ci in range(4):
                    co = co4 + ci
                    ps_p = ps_s.tile([128, TS], f32, tag="s")
                    for h in range(HPC):
                        nc.tensor.matmul(ps_p, wp_sb[:, h, co * 128:(co + 1) * 128],
                                         yts[h], start=(h == 0), stop=(h == HPC - 1))
                    if ci % 2 == 0:
                        nc.vector.tensor_copy(ot[:, ci, :], ps_p)
                    else:
                        nc.scalar.copy(ot[:, ci, :], ps_p)
                nc.sync.dma_start(
                    out=outT[co4 * 128:(co4 + 4) * 128,
                             t0:t0 + TS].rearrange("(ck p) t -> p ck t", p=128),
                    in_=ot)

    # Restrict the activation-table picker to the one set containing every
    # ACT function we use (exp, ln, square, copy, identity): without this the
    # greedy picker alternates exp_and_others <-> natural_log, inserting a
    # ~1.3us table load per switch. Set ids are positions in act_info.json's
    # list, so unwanted sets are emptied rather than removed.
    import concourse.hw_specs as hw_specs
    import concourse.bacc as bacc_mod

    orig = hw_specs.get_activation_tables

    def only_combined(arch):
        t = orig(arch)
        return {k: (v if k == "natural_log_exp_and_others" else set())
                for k, v in t.items()}

    hw_specs.get_activation_tables = only_combined
    bacc_mod.get_activation_tables = only_combined
    try:
        nc.compile()
    finally:
        hw_specs.get_activation_tables = orig
        bacc_mod.get_activation_tables = orig
    return nc


def _prep_inputs(x, ve, cos, sin, Wq, Wk, Wv, Wproj, Wgate, W):
    import ml_dtypes

    bf = ml_dtypes.bfloat16
    cosT = np.ascontiguousarray(cos[0, :, 0, :].T)  # (64, T)
    sinT = np.ascontiguousarray(sin[0, :, 0, :].T)
    cc = np.concatenate([cosT, cosT], axis=0)
    ss = np.concatenate([sinT, -sinT], axis=0)
    cs = np.ascontiguousarray(np.stack([cc, ss], axis=1)).astype(bf)  # (128,2,T)
    p = np.arange(128)[:, None]
    f = np.arange(128)[None, :]
    btri = (p <= f).astype(bf)
    etri = (f <= p + (W % 128)).astype(bf)

    in_maps = []
    for core in range(8):
        b, g = core // NKV, core % NKV
        hs = slice(g * HPC * HD, (g + 1) * HPC * HD)
        ks = slice(g * HD, (g + 1) * HD)
        in_maps.append({
            "xT": np.ascontiguousarray(x[b].T).astype(bf),
            "wqT": np.ascontiguousarray(Wq[hs, :].T).astype(bf),
            "wkT": np.ascontiguousarray(Wk[ks, :].T).astype(bf),
            "wvT": np.ascontiguousarray(Wv[ks, :].T).astype(bf),
            "wpT": np.ascontiguousarray(Wproj[:, hs].T).astype(bf),
            "wg": np.ascontiguousarray(Wgate[g][:, None]).astype(bf),
            "cs": cs,
            "ve": np.ascontiguousarray(ve[b][:, ks]).astype(bf),
            "btri": btri,
            "etri": etri,
        })
    return in_maps


def _run(inputs, trace=False):
    from concourse.bass_utils import run_bass_kernel_spmd

    x = np.asarray(inputs["x"], dtype=np.float32)
    ve = np.asarray(inputs["ve"], dtype=np.float32)
    cos = np.asarray(inputs["cos"], dtype=np.float32)
    sin = np.asarray(inputs["sin"], dtype=np.float32)
    Wq = np.asarray(inputs["Wq"], dtype=np.float32)
    Wk = np.asarray(inputs["Wk"], dtype=np.float32)
    Wv = np.asarray(inputs["Wv"], dtype=np.float32)
    Wproj = np.asarray(inputs["Wproj"], dtype=np.float32)
    Wgate = np.asarray(inputs["Wgate"], dtype=np.float32)
    W = int(inputs["window_size"])

    if W not in _compiled:
        _compiled[W] = _build(W)
    nc = _compiled[W]

    in_maps = _prep_inputs(x, ve, cos, sin, Wq, Wk, Wv, Wproj, Wgate, W)
    res = run_bass_kernel_spmd(nc, in_maps, core_ids=list(range(8)), trace=trace)

    out = np.zeros((B, T, C), dtype=np.float32)
    for core in range(8):
        b = core // NKV
        out[b] += res.results[core]["outT"].T.astype(np.float32)
    return out, res


def kernel(**inputs):
    out, _ = _run(inputs, trace=False)
    return out
